# revision 1
# baseline (speedup 1.0000x reference)
"""Trainium2 Bass kernel for nn_Attention_42700564857309.

Multi-head attention (b=2, n=64*64=4096, dim=256, attn_dim=128, 4 heads,
head_dim=32) sharded over 8 NeuronCores as one (batch, head) pair per core;
the host sums the 4 per-head partial outputs per batch element (row-parallel
Wo split), so no collectives are needed.

Per-core device kernel. All layouts are chosen so no on-device transposes of
activations are ever needed; all matmuls run in float32r (single-pass fp32,
1 column/cycle at N>=256 vs 4 for plain fp32, ~1e-4 relative rounding):
  inputs:  xT = query_b^T [256, 4096], cT = context_b^T [256, 4096]
           (pre-transposed on host so the contraction dim is on partitions),
           wq/wk = head slice of Wq/Wk replicated `pack` times along columns,
           wv [256, 32], wo [32, 256]
  qT = wq.T @ xT -> [pack*32, 4096]: `pack` stacked replicas on partitions,
       so row-packed (tile_position) S matmuls can read per-row-group slices
  kT = wk.T @ cT -> [pack*32, 4096]
  v  = cT.T @ wv -> [4096, 32] + a ones column (-> 33 wide) so the PV matmul
       also produces softmax row sums in psum row 32 for free
  Attention per 512-wide i-chunk, in groups of `pack` j-tiles (128 keys):
    S^T[j,i] = kT_jt.T @ qT   K=32 matmuls row-packed via tile_position so
               `pack` of them run concurrently in the 128x128 PE array
    P^T = exp(scale*S^T)      one ScalarE op spanning the group's psum banks
                              (scores are ~N(0,1): max-subtraction unneeded)
    pv[0:33] += v_aug_jt.T @ P^T   f32r, accumulated over all 32 j-tiles
  Row sums are transposed to per-partition layout via a tiny DRAM round-trip
  DMA (cross-partition moves are DMA territory; a K=1 transpose-matmul
  faults the device and gpsimd partition_broadcast misreads partition-32
  sources); 1/rowsum is then folded into the PSUM->SBUF copy of the
  projected output as a per-partition tensor_scalar multiply.

Scheduling: the PE executes its queue in order, so S-matmul groups are
emitted `lead` groups ahead of their exp/PV consumers (3 S psum slots),
and the q/k/v projection units are interleaved into the attention stream
with deadline-based emission instead of running as a serial prologue.
ScalarE exp (~128us busy) is the roofline; measured ~220us/iteration
sustained on hardware (~2.9e-4 max relative error vs the fp32 reference).
"""

import contextlib

import numpy as np

import concourse.bacc as bacc
import concourse.mybir as mybir
import concourse.tile as tile
from concourse import bass_utils
from concourse.bass import ts

F32 = mybir.dt.float32
F32R = mybir.dt.float32r

B, HH, WW, C = 2, 64, 64, 256
N = HH * WW              # 4096
AD = 128                 # attn_dim
HEADS = 4
D = AD // HEADS          # 32 head dim
SCALE = float(D) ** -0.5
NCORES = 8

PACK = 3                 # row-packed S^T matmuls / exp group size (psum banks)
IC = 512                 # i-chunk width (one psum bank of fp32)
NIC = N // IC            # 8 i-chunks
JT = 128                 # j-tile height
NJT = N // JT            # 32 j-tiles
NIT = IC // JT           # 4 i-tiles per chunk
VW = D + 1               # v width incl. ones column

GROUPS = [PACK] * (NJT // PACK) + ([NJT % PACK] if NJT % PACK else [])


def build_program(mm_dt=F32R, proj_dt=F32R, n_ic=NIC, n_groups=None,
                  reps=1, loop_reps=None, pack=2, s_bufs=3, lead=2, pt_bufs=3, s_dt=None, tune=False, pv2=False,
                  skip_exp=False, skip_s=False, skip_pv=False, no_pack=False,
                  skip_indma=False):
    groups_all = [pack] * (NJT // pack) + ([NJT % pack] if NJT % pack else [])
    s_dt = mm_dt if s_dt is None else s_dt
    nc = bacc.Bacc("TRN2", target_bir_lowering=False, debug=False)

    IN_DT = proj_dt
    xT_d = nc.dram_tensor("xT", [C, N], IN_DT, kind="ExternalInput")
    cT_d = nc.dram_tensor("cT", [C, N], IN_DT, kind="ExternalInput")
    wq_d = nc.dram_tensor("wq", [C, PACK * D], IN_DT, kind="ExternalInput")
    wk_d = nc.dram_tensor("wk", [C, PACK * D], IN_DT, kind="ExternalInput")
    wv_d = nc.dram_tensor("wv", [C, D], IN_DT, kind="ExternalInput")
    wo_d = nc.dram_tensor("wo", [D, C], IN_DT, kind="ExternalInput")
    out_d = nc.dram_tensor("out", [N, C], F32, kind="ExternalOutput")

    with tile.TileContext(nc) as tc:
        with tc.tile_pool(name="big", bufs=1) as big, \
             tc.tile_pool(name="pt", bufs=pt_bufs) as ptp, \
             tc.tile_pool(name="att", bufs=3 if tune else 2) as attp, \
             tc.tile_pool(name="small", bufs=6 if tune else 4) as small, \
             tc.tile_pool(name="outp", bufs=4 if tune else 3) as outp, \
             tc.tile_pool(name="spsum", bufs=s_bufs, space="PSUM") as sps_p, \
             tc.tile_pool(name="pvpsum", bufs=1, space="PSUM") as pv_p, \
             tc.tile_pool(name="oppsum", bufs=1, space="PSUM") as op_p, \
             tc.tile_pool(name="dram", bufs=3 if tune else 2, space="DRAM") as dramp:

            loop_ctx = (tc.For_i(0, loop_reps, 1) if loop_reps
                        else contextlib.nullcontext())
            with loop_ctx:
              for _rep in range(reps):
                # ---- load inputs ---------------------------------------
                xT = big.tile([128, 2, N], IN_DT, tag="xT")
                cT = big.tile([128, 2, N], IN_DT, tag="cT")
                wq = big.tile([128, 2, PACK * D], IN_DT, tag="wq")
                wk = big.tile([128, 2, PACK * D], IN_DT, tag="wk")
                wv = big.tile([128, 2, D], IN_DT, tag="wv")
                wo = big.tile([96 if pv2 else D, C], IN_DT, tag="wo")
                ones = big.tile([128, 1], F32, tag="ones")
                HN = N // 2
                for cc in range(2):
                    nc.sync.dma_start(out=wq[:, cc, :],
                                      in_=wq_d.ap()[ts(cc, 128), :])
                    nc.sync.dma_start(out=wk[:, cc, :],
                                      in_=wk_d.ap()[ts(cc, 128), :])
                    nc.sync.dma_start(out=wv[:, cc, :],
                                      in_=wv_d.ap()[ts(cc, 128), :])
                    if not skip_indma:
                        QN = N // 4 if tune else HN
                        for q0 in range(0, HN, QN):
                            nc.sync.dma_start(
                                out=xT[:, cc, q0:q0 + QN],
                                in_=xT_d.ap()[ts(cc, 128), q0:q0 + QN])
                            nc.sync.dma_start(
                                out=cT[:, cc, q0:q0 + QN],
                                in_=cT_d.ap()[ts(cc, 128), q0:q0 + QN])
                for cc in range(2):
                    if not skip_indma:
                        QN = N // 4 if tune else HN
                        for q0 in range(HN, N, QN):
                            nc.sync.dma_start(
                                out=cT[:, cc, q0:q0 + QN],
                                in_=cT_d.ap()[ts(cc, 128), q0:q0 + QN])
                            nc.sync.dma_start(
                                out=xT[:, cc, q0:q0 + QN],
                                in_=xT_d.ap()[ts(cc, 128), q0:q0 + QN])
                nc.sync.dma_start(out=wo[0:D, :], in_=wo_d.ap())
                if pv2:
                    nc.sync.dma_start(out=wo[64:64 + D, :], in_=wo_d.ap())
                nc.vector.memset(ones[:], 1.0)
                if skip_exp or skip_s or skip_pv:
                    dummyf = big.tile([128, pack * IC], F32, tag="dummyf")
                    nc.vector.memset(dummyf[:], 0.5)
                    dummyr = big.tile([128, pack * IC], mm_dt, tag="dummyr")
                    nc.vector.tensor_copy(dummyr[:], dummyf[:])

                # ---- projection units (interleaved into attention) -----
                qT = big.tile([pack * D, N], s_dt, tag="qT")
                kT = big.tile([pack * D, N], s_dt, tag="kT")
                vsb = big.tile([128, NJT, VW], mm_dt, tag="vsb")
                for jt in range(NJT):                  # preset ones column
                    nc.vector.tensor_copy(vsb[:, jt, D:VW], ones[:])

                def emit_qT_unit(ic):
                    pq = op_p.tile([pack * D, IC], F32, tag="op", name="pq")
                    nc.tensor.matmul(pq[:], lhsT=wq[:, 0, 0:pack * D],
                                     rhs=xT[:, 0, ts(ic, IC)],
                                     start=True, stop=False)
                    nc.tensor.matmul(pq[:], lhsT=wq[:, 1, 0:pack * D],
                                     rhs=xT[:, 1, ts(ic, IC)],
                                     start=False, stop=True)
                    nc.vector.tensor_copy(qT[:, ts(ic, IC)], pq[:])

                def emit_kT_unit(ic):
                    pk = op_p.tile([pack * D, IC], F32, tag="op", name="pk")
                    nc.tensor.matmul(pk[:], lhsT=wk[:, 0, 0:pack * D],
                                     rhs=cT[:, 0, ts(ic, IC)],
                                     start=True, stop=False)
                    nc.tensor.matmul(pk[:], lhsT=wk[:, 1, 0:pack * D],
                                     rhs=cT[:, 1, ts(ic, IC)],
                                     start=False, stop=True)
                    nc.vector.tensor_copy(kT[:, ts(ic, IC)], pk[:])

                def emit_v_unit(g):
                    for jt in range(pack * g, min(pack * (g + 1), NJT)):
                        pvj = op_p.tile([128, D], F32, tag="op", name="pvj")
                        nc.tensor.matmul(pvj[:],
                                         lhsT=cT[:, 0, ts(jt, JT)],
                                         rhs=wv[:, 0, :],
                                         start=True, stop=False)
                        nc.tensor.matmul(pvj[:],
                                         lhsT=cT[:, 1, ts(jt, JT)],
                                         rhs=wv[:, 1, :],
                                         start=False, stop=True)
                        nc.vector.tensor_copy(vsb[:, jt, 0:D], pvj[:])

                # ---- attention main loop (software-pipelined) ----------
                glist = []
                gsel = groups_all if n_groups is None else groups_all[:n_groups]
                njt_used = sum(gsel)
                for ic in range(n_ic):
                    jt0 = 0
                    for gs in gsel:
                        glist.append((ic, jt0, gs))
                        jt0 += gs

                sp_t, pt_t, pv_t = {}, {}, {}
                att_t, rc_t = {}, {}
                pending = []

                def emit_S(k):
                    ic, jt0, gs = glist[k]
                    sp = sps_p.tile([128, pack * IC], F32, tag="s", name="sp")
                    sp_t[k] = sp
                    for t in range(gs):
                        if skip_s:
                            continue
                        if no_pack:
                            nc.tensor.matmul(
                                sp[:, ts(t, IC)],
                                lhsT=kT[0:D, ts(jt0 + t, JT)],
                                rhs=qT[0:D, ts(ic, IC)],
                                start=True, stop=True)
                        else:
                            nc.tensor.matmul(
                                sp[:, ts(t, IC)],
                                lhsT=kT[32 * t: 32 * t + D, ts(jt0 + t, JT)],
                                rhs=qT[32 * t: 32 * t + D, ts(ic, IC)],
                                start=True, stop=True,
                                tile_position=(32 * t, 0))

                def emit_exp(k):
                    ic, jt0, gs = glist[k]
                    sp = sp_t.pop(k)
                    pt = ptp.tile([128, pack * IC], mm_dt, tag="pt", name="pt")
                    pt_t[k] = pt
                    if not skip_exp:
                        nc.scalar.activation(
                            out=pt[:, 0: gs * IC],
                            in_=(dummyf if skip_s else sp)[:, 0: gs * IC],
                            func=mybir.ActivationFunctionType.Exp,
                            scale=SCALE)

                def finalize_dve(ic):
                    pv = pv_t.pop(ic)
                    AH = 97 if pv2 else VW
                    att = attp.tile([AH, IC], proj_dt, tag="att", name="att")
                    att_t[ic] = att
                    nc.vector.tensor_copy(att[:], (dummyf[0:AH, 0:IC] if skip_pv
                                                   else pv[0:AH, :]))
                    srow = dramp.tile([2, IC], F32, tag="srow")
                    nc.sync.dma_start(out=srow[0:1, :],
                                      in_=att[D:VW, :].bitcast(F32))
                    if pv2:
                        nc.sync.dma_start(out=srow[1:2, :],
                                          in_=att[96:97, :].bitcast(F32))
                    sumsT = small.tile([128, NIT], F32, tag="sumsT")
                    nc.sync.dma_start(
                        out=sumsT[:],
                        in_=srow[0:1, :].rearrange("one (t p) -> (one p) t",
                                                   p=JT))
                    rc = small.tile([128, NIT], F32, tag="rc", name="rc")
                    rc_t[ic] = rc
                    if pv2:
                        sumsT1 = small.tile([128, NIT], F32, tag="sumsT1",
                                            name="sumsT1")
                        nc.sync.dma_start(
                            out=sumsT1[:],
                            in_=srow[1:2, :].rearrange(
                                "one (t p) -> (one p) t", p=JT))
                        nc.vector.tensor_add(sumsT[:], sumsT[:], sumsT1[:])
                    nc.vector.reciprocal(rc[:], sumsT[:])
                    for t4 in range(NIT):
                        pending.append((ic, t4))

                def emit_PV(k):
                    ic, jt0, gs = glist[k]
                    if jt0 == 0:
                        pv_t[ic] = pv_p.tile([128, IC], F32, tag="pv", name="pv")
                    pv = pv_t[ic]
                    pt = pt_t.pop(k)
                    for t in range(gs):
                        if skip_pv:
                            continue
                        jt = jt0 + t
                        if pv2:
                            base = 64 * (jt % 2)
                            nc.tensor.matmul(
                                pv[base:base + VW, :],
                                lhsT=vsb[:, jt, :],
                                rhs=(dummyr if skip_exp else pt)[:, ts(t, IC)],
                                start=(jt == 0),
                                stop=(jt == njt_used - 1),
                                tile_position=(0, base))
                        else:
                            nc.tensor.matmul(
                                pv[0:VW, :],
                                lhsT=vsb[:, jt, :],
                                rhs=(dummyr if skip_exp else pt)[:, ts(t, IC)],
                                start=(jt == 0),
                                stop=(jt == njt_used - 1))
                    if jt0 + gs == njt_used:
                        finalize_dve(ic)

                ot_t = {}

                def emit_op(ic, t4):
                    att, rc = att_t[ic], rc_t[ic]
                    op = op_p.tile([128, IC], F32, tag="op", name="op")
                    nc.tensor.matmul(op[:, 0:C],
                                     lhsT=att[0:D, ts(t4, JT)],
                                     rhs=wo[0:D, :],
                                     start=True, stop=not pv2)
                    if pv2:
                        nc.tensor.matmul(op[:, 0:C],
                                         lhsT=att[64:96, ts(t4, JT)],
                                         rhs=wo[64:96, :],
                                         start=False, stop=True,
                                         tile_position=(64, 0))
                    if t4 == 0:
                        ot_t[ic] = outp.tile([128, NIT, C], F32, tag="ot",
                                             name="ot")
                    ot = ot_t[ic]
                    nc.vector.tensor_scalar_mul(ot[:, t4, :], op[:, 0:C],
                                                rc[:, t4:t4 + 1])
                    if t4 == NIT - 1:
                        # one DMA for the whole 512-row chunk; HBM rows
                        # ic*512 + t4*128 + p  <-  sbuf [p, t4, :]
                        dst = out_d.ap()[ic * IC:(ic + 1) * IC, :].rearrange(
                            "(t p) c -> p t c", p=JT)
                        nc.sync.dma_start(out=dst, in_=ot_t.pop(ic)[:])

                nvu = (njt_used + pack - 1) // pack       # v proj units
                nku = (njt_used * JT + IC - 1) // IC      # kT proj units
                if glist:
                    emit_qT_unit(0)
                    emit_kT_unit(0)
                    emit_v_unit(0)
                    qT_done, kT_done, v_done = 1, 1, 1
                    for j in range(min(lead, len(glist))):
                        emit_S(j)
                    for k in range(len(glist)):
                        j = k + lead
                        if j < len(glist):
                            icj, jt0j, gsj = glist[j]
                            for la in (j, j + 1):
                                if la < len(glist) and glist[la][1] == 0 \
                                        and qT_done <= glist[la][0] < n_ic:
                                    emit_qT_unit(qT_done)
                                    qT_done += 1
                            need_k = min(((jt0j + gsj) * JT + IC - 1) // IC,
                                         nku) if icj == 0 else nku
                            while kT_done < need_k:
                                emit_kT_unit(kT_done)
                                kT_done += 1
                            gidx = (k + 2) if icj == 0 else nvu
                            while v_done < min(gidx, nvu):
                                emit_v_unit(v_done)
                                v_done += 1
                            emit_S(j)
                        emit_exp(k)
                        emit_PV(k)
                        if pending:
                            emit_op(*pending.pop(0))
                    while pending:
                        emit_op(*pending.pop(0))

    nc.compile()
    return nc


_CACHE = {}


def get_program():
    if "nc" not in _CACHE:
        _CACHE["nc"] = build_program()
    return _CACHE["nc"]


def make_in_maps(query, context, Wq, Wk, Wv, Wo):
    q = np.ascontiguousarray(
        np.asarray(query, dtype=np.float32).reshape(B, N, C).transpose(0, 2, 1))
    c = np.ascontiguousarray(
        np.asarray(context, dtype=np.float32).reshape(B, N, C).transpose(0, 2, 1))
    Wq = np.asarray(Wq, dtype=np.float32)
    Wk = np.asarray(Wk, dtype=np.float32)
    Wv = np.asarray(Wv, dtype=np.float32)
    Wo = np.asarray(Wo, dtype=np.float32)
    in_maps = []
    for core in range(NCORES):
        b, h = divmod(core, HEADS)
        in_maps.append({
            "xT": q[b],
            "cT": c[b],
            "wq": np.ascontiguousarray(
                np.tile(Wq[:, h * D:(h + 1) * D], (1, PACK))),
            "wk": np.ascontiguousarray(
                np.tile(Wk[:, h * D:(h + 1) * D], (1, PACK))),
            "wv": np.ascontiguousarray(Wv[:, h * D:(h + 1) * D]),
            "wo": np.ascontiguousarray(Wo[h * D:(h + 1) * D, :]),
        })
    return in_maps


def combine(results):
    out = np.zeros((B, N, C), np.float32)
    for core in range(NCORES):
        b = core // HEADS
        out[b] += results[core]["out"]
    return out.reshape(B, HH, WW, C)


def kernel(query, context, Wq, Wk, Wv, Wo):
    nc = get_program()
    in_maps = make_in_maps(query, context, Wq, Wk, Wv, Wo)
    res = bass_utils.run_bass_kernel_spmd(nc, in_maps,
                                          core_ids=list(range(NCORES)))
    return combine(res.results)



# revision 2
# speedup vs baseline: 1.1146x; 1.1146x over previous
"""Trainium2 Bass kernel for nn_Attention_42700564857309 (v3).

Multi-head attention (b=2, n=4096, dim=256, attn_dim=128, 4 heads,
head_dim=32), one (batch, head) pair per NeuronCore; host divides the raw
per-head outputs by the softmax row-sums (normalization commutes through Wo)
and sums the 4 heads per batch element, so no collectives are needed.

Per-core structure (all matmul operands bf16, PSUM accumulates f32):
  - qT/kT = 3 stacked replicas of the head projection so S^T matmuls are
    3-way row-packed via tile_position (3 concurrent K=32 matmuls).
  - S^T j-tiles [128 keys, 512 queries] stream through a flat 6-bank PSUM
    ring; softmax exp is split across two engines per 6-tile period:
      ScalarE:  P = exp(scale*S) over ring cols [0, b)       (bf16 out)
      VectorE:  Schraudolph fast-exp over cols [b, 3072):
                int16 bits = round(S*(scale*128/ln2) + (16256-5.498)),
                bitcast bf16 (one tensor_scalar mult+add, ~3% max rel err;
                softmax normalization cancels most of it).
    Chunks are <= 3 ring slots so their latency fits inside the PE work
    available between the ring write-after-read dependencies (a 4-slot
    chunk measurably ping-pongs the whole pipeline into serial execution).
  - PV accumulates [33, 512] per i-chunk over 32 j-tiles (ones column in
    vsb row 32 produces row sums for free); att evacuated bf16 by ScalarE;
    out = att @ Wo written bf16 straight to HBM, normalized on host.
  - PSUM->SBUF evacuation is split for balance: att/op/v copies on
    ScalarE, qT/kT copies on VectorE.
  - tc.For_i puts an all-engine barrier between iterations, so the body
    unrolls TWO reps with double-buffered inputs: each half prefetches the
    other half's input DMAs mid-emission, amortizing ramp/drain/barrier.
"""

import contextlib

import numpy as np

import concourse.bacc as bacc
import concourse.mybir as mybir
import concourse.tile as tile
from concourse import bass_utils
from concourse.bass import ts
from concourse.bass import _add_dep_helper

F32 = mybir.dt.float32
BF16 = mybir.dt.bfloat16
I16 = mybir.dt.int16

B, HH, WW, C = 2, 64, 64, 256
N = HH * WW              # 4096
AD = 128                 # attn_dim
HEADS = 4
D = AD // HEADS          # 32 head dim
SCALE = float(D) ** -0.5
NCORES = 8
PACK = 3                 # S^T row-pack width (tile positions 0/32/64)

IC = 512                 # i-chunk width (one psum bank of fp32)
NIC = N // IC            # 8 i-chunks
JT = 128                 # j-tile height
NJT = N // JT            # 32 j-tiles per i-chunk
NU = NIC * NJT           # 256 (ic, jt) units
RING = 6                 # psum ring slots (banks) for S^T
PTS = 12                 # pt ring slots (sbuf)
LN2 = float(np.log(2.0))


def build_program(loop_reps=None, b_cols=1536, lag_pv=8, unroll=2,
                  skip_exp_act=False, skip_exp_dve=False, skip_s=False,
                  skip_pv=False, skip_proj=False, skip_op=False,
                  evac_qk_act=False, evac_out_dve=False, follow_dve=False,
                  prio_exp=0, s_cols=IC, impose=None):
    nc = bacc.Bacc("TRN2", target_bir_lowering=False, debug=False)

    xT_d = nc.dram_tensor("xT", [C, N], BF16, kind="ExternalInput")
    cT_d = nc.dram_tensor("cT", [C, N], BF16, kind="ExternalInput")
    wq_d = nc.dram_tensor("wq", [C, PACK * D], BF16, kind="ExternalInput")
    wk_d = nc.dram_tensor("wk", [C, PACK * D], BF16, kind="ExternalInput")
    wv_d = nc.dram_tensor("wv", [C, D], BF16, kind="ExternalInput")
    wo_d = nc.dram_tensor("wo", [D, C], BF16, kind="ExternalInput")
    out_d = nc.dram_tensor("out", [N, C], BF16, kind="ExternalOutput")
    den_d = nc.dram_tensor("den", [NIC, IC], BF16, kind="ExternalOutput")

    # Schraudolph constants: bf16 bits = round(s * SA + SB)
    SA = SCALE * 128.0 / LN2
    SB = 16256.0 - 5.498

    QP = PACK * D            # qT/kT partition count (96)

    with tile.TileContext(nc) as tc:
        with tc.tile_pool(name="big", bufs=1) as big, \
             tc.tile_pool(name="attp", bufs=2) as attp, \
             tc.tile_pool(name="otp", bufs=2) as otp, \
             tc.tile_pool(name="ring", bufs=1, space="PSUM") as ringp, \
             tc.tile_pool(name="pvp", bufs=1, space="PSUM") as pvp, \
             tc.tile_pool(name="miscp", bufs=1, space="PSUM") as miscp:

            nbuf = 2 if (loop_reps and unroll == 2) else 1

            def in_set(i):
                s = str(i)
                shapes = dict(xT=[128, 2, N], cT=[128, 2, N],
                              wq=[128, 2, QP], wk=[128, 2, QP],
                              wv=[128, 2, D], wo=[D, C],
                              vsb=[128, NJT, 34])
                return {k: big.tile(sh, BF16, tag=k + s, name=k + s)
                        for k, sh in shapes.items()}

            sets = [in_set(i) for i in range(nbuf)]
            qT = big.tile([QP, N], BF16, tag="qT")
            kT = big.tile([QP, N], BF16, tag="kT")
            # separate per-engine P tiles: a shared tile would create
            # conservative cross-engine WAW deps (the int16 bitcast defeats
            # sub-tile range tracking) that serialize ScalarE vs VectorE
            pt_a = big.tile([128, 6 * IC], BF16, tag="pt_a")
            pt_d = big.tile([128, 6 * IC], BF16, tag="pt_d")
            # two PSUM ring tiles (3 banks each): PSUM hazards are tracked
            # ~whole-tile, so one flat ring serializes S-matmuls against
            # BOTH exp engines; per-engine tiles make the coarse tracking
            # coincide with the true chunk-granular dependency structure.
            ringA = ringp.tile([128, 3 * IC], F32, tag="ringA")
            ringD = ringp.tile([128, 3 * IC], F32, tag="ringD")
            pv = pvp.tile([128, IC], F32, tag="pv")

            def load_inputs(T, which=("w", "c", "x")):
                if "w" in which:
                    for cc in range(2):
                        nc.sync.dma_start(out=T["wq"][:, cc, :],
                                          in_=wq_d.ap()[ts(cc, 128), :])
                        nc.sync.dma_start(out=T["wk"][:, cc, :],
                                          in_=wk_d.ap()[ts(cc, 128), :])
                        nc.sync.dma_start(out=T["wv"][:, cc, :],
                                          in_=wv_d.ap()[ts(cc, 128), :])
                    nc.sync.dma_start(out=T["wo"][:], in_=wo_d.ap())
                for src_d, key in ((cT_d, "c"), (xT_d, "x")):
                    if key in which:
                        dst = T["cT" if key == "c" else "xT"]
                        for q in range(4):
                            for cc in range(2):
                                nc.sync.dma_start(
                                    out=dst[:, cc, ts(q, N // 4)],
                                    in_=src_d.ap()[ts(cc, 128), ts(q, N // 4)])

            def load_chunk(T, key, q):
                src_d = cT_d if key == "c" else xT_d
                dst = T["cT" if key == "c" else "xT"]
                for cc in range(2):
                    nc.sync.dma_start(
                        out=dst[:, cc, ts(q, N // 4)],
                        in_=src_d.ap()[ts(cc, 128), ts(q, N // 4)])

            # ---- preamble: first-rep inputs + ones columns -------------
            load_inputs(sets[0])
            for T in sets:
                nc.vector.memset(T["vsb"][:, :, D:D + 1], 1.0)
                nc.vector.memset(T["vsb"][:, :, D + 1:D + 2], 0.0)
            if skip_exp_act or skip_exp_dve:
                nc.vector.memset(pt_a[:], 0.25)
                nc.vector.memset(pt_d[:], 0.25)
            if skip_s:
                nc.vector.memset(ringA[:], 0.125)
                nc.vector.memset(ringD[:], 0.125)
            if skip_proj:
                nc.vector.memset(qT[:], 0.5)
                nc.vector.memset(kT[:], 0.5)
                for T in sets:
                    nc.vector.memset(T["vsb"][:, :, 0:D], 0.5)

            # ---- per-rep emission --------------------------------------
            def emit_rep(T, TP=None):
                """Emit one attention rep using input set T; if TP is
                given, prefetch its inputs mid-body for the next rep."""
                xT, cT = T["xT"], T["cT"]
                wq, wk, wv, wo, vsb = (T["wq"], T["wk"], T["wv"], T["wo"],
                                       T["vsb"])

                def emit_proj_unit(j, which, bank=None):
                    # qT/kT cols [512j, 512j+512); bank="A"/"D" borrows a
                    # ring psum slot (parallel ramp chains at rep start)
                    if skip_proj:
                        return
                    w_t, x_t, dst = ((wq, xT, qT) if which == "q"
                                     else (wk, cT, kT))
                    if bank == "A":
                        p = ringA[0:QP, 0:IC]
                    elif bank == "D":
                        p = ringD[0:QP, 0:IC]
                    else:
                        p = miscp.tile([QP, IC], F32, tag="m",
                                       name="p" + which)[:]
                    nc.tensor.matmul(p, lhsT=w_t[:, 0, :],
                                     rhs=x_t[:, 0, ts(j, IC)],
                                     start=True, stop=False)
                    nc.tensor.matmul(p, lhsT=w_t[:, 1, :],
                                     rhs=x_t[:, 1, ts(j, IC)],
                                     start=False, stop=True)
                    nc.vector.tensor_copy(dst[:, ts(j, IC)], p)

                def emit_v_quad(g):
                    if skip_proj:
                        return
                    pvq = miscp.tile([128, 4, D], F32, tag="m", name="pvq")
                    for e in range(4):
                        jt = 4 * g + e
                        nc.tensor.matmul(pvq[:, e, :],
                                         lhsT=cT[:, 0, ts(jt, JT)],
                                         rhs=wv[:, 0, :],
                                         start=True, stop=False)
                        nc.tensor.matmul(pvq[:, e, :],
                                         lhsT=cT[:, 1, ts(jt, JT)],
                                         rhs=wv[:, 1, :],
                                         start=False, stop=True)
                    nc.scalar.copy(vsb[:, 4 * g:4 * g + 4, 0:D], pvq[:])

                def emit_S_pack(u0):
                    if skip_s:
                        return
                    rtile = ringA if u0 % 6 < 3 else ringD
                    for t in range(PACK):
                        u = u0 + t
                        if u >= NU:
                            break
                        ic, jt = divmod(u, NJT)
                        r = 32 * t
                        nc.tensor.matmul(
                            rtile[:, u % 3 * IC:u % 3 * IC + s_cols],
                            lhsT=kT[r:r + D, ts(jt, JT)],
                            rhs=qT[r:r + D, ic * IC:ic * IC + s_cols],
                            start=True, stop=True,
                            tile_position=(r, 0))

                def emit_exp_act(w):    # ring cols [0, b_cols)
                    if skip_exp_act:
                        return
                    e = min(b_cols, (NU - w) * IC)
                    s = ((w // 6) * 3 % 6) * IC
                    ctx = (tc.high_priority(offset=prio_exp) if prio_exp
                           else contextlib.nullcontext())
                    with ctx:
                        _emit_exp_act_inner(e, s)

                def _emit_exp_act_inner(e, s):
                    nc.scalar.activation(
                        out=pt_a[:, s:s + e],
                        in_=ringA[:, 0:e],
                        func=mybir.ActivationFunctionType.Exp,
                        scale=SCALE)

                def emit_exp_dve(w):    # ring cols [b_cols, 3072)
                    if skip_exp_dve:
                        return
                    e = min(RING * IC, (NU - w) * IC)
                    if e <= b_cols:
                        return
                    s = ((w // 6) * 3 % 6) * IC
                    ctx = (tc.high_priority(offset=prio_exp) if prio_exp
                           else contextlib.nullcontext())
                    with ctx:
                        ins = _emit_exp_dve_inner(e, s)

                def _emit_exp_dve_inner(e, s):
                    return nc.vector.tensor_scalar(
                        pt_d[:, s:s + e - b_cols].bitcast(I16),
                        ringD[:, 0:e - b_cols],
                        SA, SB,
                        mybir.AluOpType.mult, mybir.AluOpType.add)
                    if follow_dve and w == 120:
                        tile.tile_follow(ins, log_all_deps=True)
                    return ins

                def emit_PV(u):
                    if skip_pv:
                        return
                    ic, jt = divmod(u, NJT)
                    half = u % 6 < 3
                    slot = ((u // 6) * 3 + u % 3) % 6
                    src_t = pt_a if half else pt_d
                    nc.tensor.matmul(
                        pv[0:D + 1, :],
                        lhsT=vsb[:, jt, 0:D + 1],
                        rhs=src_t[:, ts(slot, IC)],
                        start=(jt == 0), stop=(jt == NJT - 1))

                att_t, ot_t = {}, {}

                def emit_att(ic):
                    if skip_pv:
                        return
                    att = attp.tile([34, IC], BF16, tag="att", name="att")
                    att_t[ic] = att
                    cp = nc.vector.tensor_copy if evac_out_dve \
                        else nc.scalar.copy
                    cp(att[0:D + 1, :], pv[0:D + 1, :])
                    nc.sync.dma_start(out=den_d.ap()[ic:ic + 1, :],
                                      in_=att[D:D + 1, :])

                def emit_op_pair(ic, p):
                    if skip_op or skip_pv:
                        return
                    att = att_t[ic]
                    opp = miscp.tile([128, 2, C], F32, tag="m", name="opp")
                    for e in range(2):
                        t4 = 2 * p + e
                        nc.tensor.matmul(opp[:, e, :],
                                         lhsT=att[0:D, ts(t4, JT)],
                                         rhs=wo[:], start=True, stop=True)
                    if p == 0:
                        ot_t[ic] = otp.tile([128, 4, C], BF16, tag="ot",
                                            name="ot")
                    ot = ot_t[ic]
                    cp = nc.vector.tensor_copy if evac_out_dve \
                        else nc.scalar.copy
                    cp(ot[:, 2 * p:2 * p + 2, :], opp[:])
                    if p == 1:
                        dst = out_d.ap()[ts(ic, IC), :].rearrange(
                            "(t p) c -> p t c", p=JT)
                        nc.sync.dma_start(out=dst, in_=ot_t.pop(ic)[:])

                # ---- schedule ------------------------------------------
                PRIO = {"ldw": 0, "ldc": 0, "ldx": 0, "proj": 1, "v": 1,
                        "exp_act": 2, "exp_dve": 3, "S": 4, "att": 5,
                        "PV": 6, "op": 7}
                n_au = (b_cols + IC - 1) // IC      # units in ACT chunk
                sched = [[] for _ in range(NU + 16)]

                def add(u, item):
                    sched[min(max(u, 0), len(sched) - 1)].append(item)

                for u in range(NU):
                    if u % PACK == 0:
                        add(u, ("S", u))
                    if u % 6 == 0:
                        add(u + n_au, ("exp_act", u))
                        add(u + 6, ("exp_dve", u))
                    add(u + lag_pv, ("PV", u))
                add(0, ("proj", 0, "k"))
                add(0, ("proj", 0, "q"))
                add(1, ("proj", 1, "k"))
                add(3, ("proj", 2, "k"))
                for j in range(3, 8):
                    add(4 * j - 8, ("proj", j, "k"))
                for j in range(1, 8):
                    add(32 * j - 24, ("proj", j, "q"))
                for g in range(8):
                    add(4 * g + 2, ("v", g))
                for ic in range(NIC):
                    u9 = 32 * ic + 31 + lag_pv
                    add(u9 + 1, ("att", ic))
                    add(u9 + 3, ("op", ic, 0))
                    add(u9 + 5, ("op", ic, 1))
                if TP is not None:
                    add(6, ("ldw",))
                    for q in range(4):
                        add(8 + 3 * q, ("ldc", q))
                        add(20 + 3 * q, ("ldx", q))

                emits = {
                    "S": emit_S_pack, "exp_act": emit_exp_act,
                    "exp_dve": emit_exp_dve, "PV": emit_PV,
                    "proj": emit_proj_unit, "v": emit_v_quad,
                    "att": emit_att, "op": emit_op_pair,
                    "ldw": lambda: load_inputs(TP, which=("w",)),
                    "ldc": lambda q: load_chunk(TP, "c", q),
                    "ldx": lambda q: load_chunk(TP, "x", q),
                }
                for ops in sched:
                    for item in sorted(ops, key=lambda it: PRIO[it[0]]):
                        emits[item[0]](*item[1:])

            # ---- body ---------------------------------------------------
            R = loop_reps or 1
            if R >= 2 and unroll == 2:
                with tc.For_i(0, R // 2, 1):
                    emit_rep(sets[0], TP=sets[1])
                    emit_rep(sets[1], TP=sets[0])
                if R % 2:
                    emit_rep(sets[0])
            elif R >= 2:
                with tc.For_i(0, R, 1):
                    emit_rep(sets[0], TP=sets[0])
            else:
                emit_rep(sets[0])

            if impose:
                # pin per-engine instruction order (copied from a build
                # whose cost model saw realistic packed-matmul costs);
                # same-engine chains cost nothing at runtime (FIFO).
                im = nc.inst_map
                for e, seq in impose.items():
                    prev = None
                    for name in seq:
                        cur = im.get(name)
                        if cur is None or str(cur.engine) != e:
                            continue

                        if prev is not None:
                            _add_dep_helper(cur, prev, sync=True,
                                            reason="imposed order")
                        prev = cur

    nc.compile()
    return nc


def build_scheduled(loop_reps=None, **kw):
    """Two-phase build: phase 1 with narrow S-matmuls so the scheduler's
    cost model matches the real (tile_position-packed) S cost, then the
    real program with phase 1's per-engine order imposed."""
    nc1 = build_program(loop_reps=loop_reps, s_cols=171, **kw)
    KEEP = {"InstMatmult", "InstActivation", "InstTensorScalarPtr",
            "InstTensorCopy", "InstDMACopy", "InstMemset"}
    order = {}
    for b in nc1.m.functions[0].blocks:
        for i in b.instructions:
            if type(i).__name__ in KEEP:
                order.setdefault(str(i.engine), []).append(i.name)
    return build_program(loop_reps=loop_reps, impose=order, **kw)


_CACHE = {}


def get_program():
    if "nc" not in _CACHE:
        _CACHE["nc"] = build_program()
    return _CACHE["nc"]


def _bf16(x):
    import ml_dtypes
    return np.asarray(x, dtype=np.float32).astype(ml_dtypes.bfloat16)


def make_in_maps(query, context, Wq, Wk, Wv, Wo):
    q = np.ascontiguousarray(
        np.asarray(query, np.float32).reshape(B, N, C).transpose(0, 2, 1))
    c = np.ascontiguousarray(
        np.asarray(context, np.float32).reshape(B, N, C).transpose(0, 2, 1))
    Wq = np.asarray(Wq, np.float32)
    Wk = np.asarray(Wk, np.float32)
    Wv = np.asarray(Wv, np.float32)
    Wo = np.asarray(Wo, np.float32)
    in_maps = []
    for core in range(NCORES):
        b, h = divmod(core, HEADS)
        in_maps.append({
            "xT": _bf16(q[b]),
            "cT": _bf16(c[b]),
            "wq": _bf16(np.tile(Wq[:, h * D:(h + 1) * D], (1, PACK))),
            "wk": _bf16(np.tile(Wk[:, h * D:(h + 1) * D], (1, PACK))),
            "wv": _bf16(Wv[:, h * D:(h + 1) * D]),
            "wo": _bf16(Wo[h * D:(h + 1) * D, :]),
        })
    return in_maps


def combine(results):
    out = np.zeros((B, N, C), np.float32)
    for core in range(NCORES):
        b = core // HEADS
        raw = np.asarray(results[core]["out"], dtype=np.float32)
        den = np.asarray(results[core]["den"], dtype=np.float32).reshape(N)
        out[b] += raw / den[:, None]
    return out.reshape(B, HH, WW, C)


def kernel(query, context, Wq, Wk, Wv, Wo):
    nc = get_program()
    in_maps = make_in_maps(query, context, Wq, Wk, Wv, Wo)
    res = bass_utils.run_bass_kernel_spmd(nc, in_maps,
                                          core_ids=list(range(NCORES)))
    return combine(res.results)
